# revision 10
# baseline (speedup 1.0000x reference)
"""Trainium2 Bass kernel for continuous-filter convolution (gnn message passing).

Reference computation (shapes hardcoded):
    features [2,256,32] f32, geometry [2,256,3] f32, centers [16] f32,
    kernel_w [16,32,32] f32, n_norm scalar
    d[z,a,b]   = sqrt(sum_c (g[z,b,c]-g[z,a,c])^2 + 1e-9)
    rbf        = exp(-10*(d[...,None]-centers)^2)            [z,a,b,n]
    k          = einsum('zabn,nij->zabij', rbf, kernel_w) / sqrt(n_norm)
    out[z,a,i] = einsum('zabij,zbj->zai', k, features)

Key restructuring: k is never materialized. Instead
    m[z,b,n,i]  = sum_j kernel_w[n,i,j] * features[z,b,j] / sqrt(n_norm)
    out[z,a,i]  = sum_{b,n} rbf[z,a,b,n] * m[z,b,n,i]
which is a [64 x 4096] @ [4096 x 32] contraction per (z, a-slice).

Sharding: 8 cores = 2 batches x 4 a-slices of 64 points. Each core gets its
geometry slice plus replicated features/weights; no cross-device reduction.

Per-core dataflow (b = point index, 2 chunks of 128 on partitions):
    d[b,(ch,a)]  direct-form distances on DVE (keeps the a==b diagonal exact)
    rbf[b,(n,a)] per (chunk, n-half) unit: DVE sub -> DVE/GpSimd square ->
                 ACT exp -> 8 accumulating PE matmuls (K=128)
    m[b,(n,i)]   two PE matmuls vs pre-transposed weights; PSUM->SBUF copies
                 split between ACT and DVE per n-half, placed off the
                 critical chain
"""

import numpy as np
from contextlib import ExitStack

import concourse.bass as bass
import concourse.tile as tile
from concourse import mybir
from concourse.bass_utils import run_bass_kernel_spmd

GAMMA = 10.0
EPS = 1e-9
B, P, C = 2, 256, 32
NB, I, J = 16, 32, 32
NCORES = 8
AS = NCORES // B  # a-slices per batch = 4
AL = P // AS      # points per a-slice = 64
NH = NB // 2      # n-half size = 8

f32 = mybir.dt.float32

# packed input A: [128, 215] = gab(192, broadcast) | gb6(6) | cb(16) | eps(1)
GA0, GB0, CB0, EPS0 = 0, 192, 198, 214
WA = 215
# packed input B: [32, 768] = ft(256) | wt(512)
FT0, WT0 = 0, 256
WB = 768


def _split_multi_waits(nc):
    """This walrus build only lowers one sync wait per instruction; Tile's
    scheduler attaches several to some instructions (notably the tail drain).
    Hoist extras into single-wait EventSemaphore instructions just before, on
    the same engine — semantically identical, sequencer waits then issues."""
    n = 0
    for fn in nc.m.functions:
        for bb in fn.blocks:
            insts = list(bb.instructions)
            new = []
            for inst in insts:
                si = getattr(inst, "sync_info", None)
                if si is not None and si.on_wait and len(si.on_wait) > 1:
                    waits = list(si.on_wait)
                    for w in waits[:-1]:
                        n += 1
                        new.append(
                            mybir.InstEventSemaphore(
                                name=f"I-msplit{n}",
                                engine=inst.engine,
                                sync_info=mybir.SyncInfo(on_wait=[w], on_update=[]),
                            )
                        )
                    inst.sync_info = mybir.SyncInfo(
                        on_wait=[waits[-1]], on_update=list(si.on_update or [])
                    )
                new.append(inst)
            try:
                bb.instructions = new
            except Exception:
                bb.instructions.clear()
                for i in new:
                    bb.add_instruction(i)
    return n


def _build_program():
    nc = bass.Bass(debug=False)
    g_a = nc.declare_dram_parameter("ina", [128, WA], f32, isOutput=False)
    g_b = nc.declare_dram_parameter("inb", [J, WB], f32, isOutput=False)
    g_out = nc.declare_dram_parameter("out", [AL, I], f32, isOutput=True)

    Act = mybir.ActivationFunctionType
    Alu = mybir.AluOpType
    const0 = nc.const_aps.aps[(f32, 0.0)]

    with ExitStack() as ctx:
        tc = ctx.enter_context(tile.TileContext(nc))
        pool = ctx.enter_context(tc.tile_pool(name="sb", bufs=1))
        pipe = ctx.enter_context(tc.tile_pool(name="pipe", bufs=2))
        ppool = ctx.enter_context(tc.tile_pool(name="ps", bufs=1, space="PSUM"))

        # warm the ACT sqrt table while DMAs are in flight (exp's table loads
        # in the idle window between the sqrt and the first exp)
        junk = pool.tile([128, 1], f32, tag="junk")
        nc.scalar.activation(junk[:], const0[:, 0:1], Act.Sqrt)

        t_a = pool.tile([128, WA], f32, tag="ina")
        nc.sync.dma_start(t_a[:], g_a[:])
        t_b = pool.tile([J, WB], f32, tag="inb")
        nc.scalar.dma_start(t_b[:], g_b[:])

        ga3 = t_a[:, GA0 : GA0 + AL * 3].rearrange("p (a c) -> p a c", c=3)
        gb3 = t_a[:, GB0 : GB0 + 6].rearrange("p (h c) -> p h c", c=3)
        cb = t_a[:, CB0 : CB0 + NB]
        epsc = t_a[:, EPS0 : EPS0 + 1]

        # distance chain, both chunks fused: d[b, (ch, a)]
        diff = pool.tile([128, 2 * AL * 3], f32, tag="diff")
        diff3 = diff[:].rearrange("p (h a c) -> p h a c", h=2, c=3)
        nc.vector.tensor_sub(
            diff3,
            ga3.unsqueeze(1).broadcast_to([128, 2, AL, 3]),
            gb3.unsqueeze(2).broadcast_to([128, 2, AL, 3]),
        )
        sqd = pool.tile([128, 2 * AL * 3], f32, tag="sqd")
        sqd3 = sqd[:].rearrange("p (h a c) -> p h a c", h=2, c=3)
        nc.vector.tensor_mul(sqd3, diff3, diff3)
        d2 = pool.tile([128, 2 * AL], f32, tag="d2")
        nc.vector.tensor_reduce(
            d2[:].rearrange("p (h a) -> p h a", h=2), sqd3, axis=mybir.AxisListType.X,
            op=Alu.add,
        )
        d = pool.tile([128, 2 * AL], f32, tag="d")
        nc.scalar.activation(d[:], d2[:], Act.Sqrt, bias=epsc)

        # m[b, (n,i)] per chunk
        pm = []
        for ch in range(2):
            p = ppool.tile([128, NB * I], f32, tag=f"pm{ch}", name=f"pm{ch}")
            nc.tensor.matmul(
                p[:],
                lhsT=t_b[:, FT0 + ch * 128 : FT0 + (ch + 1) * 128],
                rhs=t_b[:, WT0 : WT0 + NB * I],
                start=True,
                stop=True,
            )
            pm.append(p)
        t_m = [
            pool.tile([128, NB * I], f32, tag=f"m{ch}", name=f"m{ch}")
            for ch in range(2)
        ]
        HW = NH * I  # columns per n-half = 256

        # rbf + contraction, pipelined in 4 units of (chunk, n-half).
        # PSUM->SBUF m copies are interleaved: each unit's m half is copied
        # just before it is needed, alternating DVE/ACT.
        po = ppool.tile([AL, I], f32, tag="po")
        first = True
        for ch in range(2):
            for h in range(2):
                u = ch * 2 + h
                tt = pipe.tile([128, NH * AL], f32, tag="tt")
                nc.vector.tensor_sub(
                    tt[:].rearrange("p (n a) -> p n a", n=NH),
                    d[:, ch * AL : (ch + 1) * AL]
                    .unsqueeze(1)
                    .broadcast_to([128, NH, AL]),
                    cb[:, h * NH : (h + 1) * NH]
                    .unsqueeze(2)
                    .broadcast_to([128, NH, AL]),
                )
                sq2 = pipe.tile([128, NH * AL], f32, tag="sq2")
                if u in (1, 3):
                    nc.gpsimd.tensor_mul(sq2[:], tt[:], tt[:])
                else:
                    nc.vector.tensor_mul(sq2[:], tt[:], tt[:])
                # copy this unit's m half on ACT just before its exp — ACT is
                # idle there, and DVE placement delayed the first contraction
                nc.scalar.copy(
                    t_m[ch][:, h * HW : (h + 1) * HW],
                    pm[ch][:, h * HW : (h + 1) * HW],
                )
                rbf = pipe.tile([128, NH * AL], f32, tag="rbf")
                nc.scalar.activation(rbf[:], sq2[:], Act.Exp, scale=-GAMMA)
                for k in range(NH):
                    n = h * NH + k
                    nc.tensor.matmul(
                        po[:],
                        lhsT=rbf[:, k * AL : (k + 1) * AL],
                        rhs=t_m[ch][:, n * I : (n + 1) * I],
                        start=first,
                        stop=(ch == 1 and n == NB - 1),
                    )
                    first = False
        t_o = pool.tile([AL, I], f32, tag="o")
        nc.vector.tensor_copy(t_o[:], po[:])
        nc.sync.dma_start(g_out[:], t_o[:])

    _split_multi_waits(nc)
    return nc


_NC = None


def _pack_inputs(features, geometry, centers, kernel_w, n_norm):
    features = np.asarray(features, np.float32)
    geometry = np.asarray(geometry, np.float32)
    centers = np.asarray(centers, np.float32)
    kernel_w = np.asarray(kernel_w, np.float32)
    scale = 1.0 / np.sqrt(float(np.asarray(n_norm).item()))

    wt = np.ascontiguousarray(kernel_w.transpose(2, 0, 1).reshape(J, NB * I))
    in_maps = []
    for core in range(NCORES):
        z, sl = divmod(core, AS)
        ina = np.empty((128, WA), np.float32)
        ina[:, GA0 : GA0 + AL * 3] = geometry[z, sl * AL : (sl + 1) * AL, :].reshape(
            1, AL * 3
        )
        ina[:, GB0 : GB0 + 6] = (
            geometry[z].reshape(2, 128, 3).transpose(1, 0, 2).reshape(128, 6)
        )
        ina[:, CB0 : CB0 + NB] = centers.reshape(1, NB)
        ina[:, EPS0] = EPS
        inb = np.empty((J, WB), np.float32)
        inb[:, FT0 : FT0 + P] = features[z].T * scale
        inb[:, WT0 : WT0 + NB * I] = wt
        in_maps.append({"ina": ina, "inb": inb})
    return in_maps


def kernel(features, geometry, centers, kernel_w, n_norm):
    global _NC
    if _NC is None:
        _NC = _build_program()

    in_maps = _pack_inputs(features, geometry, centers, kernel_w, n_norm)
    res = run_bass_kernel_spmd(_NC, in_maps, list(range(NCORES)))

    out = np.empty((B, P, I), np.float32)
    for core in range(NCORES):
        z, sl = divmod(core, AS)
        out[z, sl * AL : (sl + 1) * AL, :] = res.results[core]["out"]
    return out


# revision 11
# speedup vs baseline: 1.0465x; 1.0465x over previous
"""Trainium2 Bass kernel for continuous-filter convolution (gnn message passing).

Reference computation (shapes hardcoded):
    features [2,256,32] f32, geometry [2,256,3] f32, centers [16] f32,
    kernel_w [16,32,32] f32, n_norm scalar
    d[z,a,b]   = sqrt(sum_c (g[z,b,c]-g[z,a,c])^2 + 1e-9)
    rbf        = exp(-10*(d[...,None]-centers)^2)            [z,a,b,n]
    k          = einsum('zabn,nij->zabij', rbf, kernel_w) / sqrt(n_norm)
    out[z,a,i] = einsum('zabij,zbj->zai', k, features)

Key restructuring: k is never materialized. Instead
    m[z,b,n,i]  = sum_j kernel_w[n,i,j] * features[z,b,j] / sqrt(n_norm)
    out[z,a,i]  = sum_{b,n} rbf[z,a,b,n] * m[z,b,n,i]
which is a [64 x 4096] @ [4096 x 32] contraction per (z, a-slice).

Sharding: 8 cores = 2 batches x 4 a-slices of 64 points. Each core gets its
geometry slice plus replicated features/weights; no cross-device reduction.

Per-core dataflow (b = point index, 2 chunks of 128 on partitions):
    d[b,(ch,a)]  direct-form distances on DVE (keeps the a==b diagonal exact)
    rbf[b,(n,a)] per (chunk, n-half) unit: DVE sub -> DVE/GpSimd square ->
                 ACT exp -> 8 accumulating PE matmuls (K=128)
    m[b,(n,i)]   two PE matmuls vs pre-transposed weights; PSUM->SBUF copies
                 split between ACT and DVE per n-half, placed off the
                 critical chain
"""

import numpy as np
from contextlib import ExitStack

import concourse.bass as bass
import concourse.tile as tile
from concourse import mybir
from concourse.bass_utils import run_bass_kernel_spmd

GAMMA = 10.0
EPS = 1e-9
B, P, C = 2, 256, 32
NB, I, J = 16, 32, 32
NCORES = 8
AS = NCORES // B  # a-slices per batch = 4
AL = P // AS      # points per a-slice = 64
NH = NB // 2      # n-half size = 8

f32 = mybir.dt.float32

# packed input A: [128, 215] = gab(192, broadcast) | gb6(6) | cb(16) | eps(1)
GA0, GB0, CB0, EPS0 = 0, 192, 198, 214
WA = 215
# packed input B: [32, 768] = ft(256) | wt(512)
FT0, WT0 = 0, 256
WB = 768


def _split_multi_waits(nc):
    """This walrus build only lowers one sync wait per instruction; Tile's
    scheduler attaches several to some instructions (notably the tail drain).
    Hoist extras into single-wait EventSemaphore instructions just before, on
    the same engine — semantically identical, sequencer waits then issues."""
    n = 0
    for fn in nc.m.functions:
        for bb in fn.blocks:
            insts = list(bb.instructions)
            new = []
            for inst in insts:
                si = getattr(inst, "sync_info", None)
                if si is not None and si.on_wait and len(si.on_wait) > 1:
                    waits = list(si.on_wait)
                    for w in waits[:-1]:
                        n += 1
                        new.append(
                            mybir.InstEventSemaphore(
                                name=f"I-msplit{n}",
                                engine=inst.engine,
                                sync_info=mybir.SyncInfo(on_wait=[w], on_update=[]),
                            )
                        )
                    inst.sync_info = mybir.SyncInfo(
                        on_wait=[waits[-1]], on_update=list(si.on_update or [])
                    )
                new.append(inst)
            try:
                bb.instructions = new
            except Exception:
                bb.instructions.clear()
                for i in new:
                    bb.add_instruction(i)
    return n


def _build_program():
    nc = bass.Bass(debug=False)
    g_a = nc.declare_dram_parameter("ina", [128, WA], f32, isOutput=False)
    g_b = nc.declare_dram_parameter("inb", [J, WB], f32, isOutput=False)
    g_out = nc.declare_dram_parameter("out", [AL, I], f32, isOutput=True)

    Act = mybir.ActivationFunctionType
    Alu = mybir.AluOpType
    const0 = nc.const_aps.aps[(f32, 0.0)]

    with ExitStack() as ctx:
        tc = ctx.enter_context(tile.TileContext(nc))
        pool = ctx.enter_context(tc.tile_pool(name="sb", bufs=1))
        pipe = ctx.enter_context(tc.tile_pool(name="pipe", bufs=2))
        ppool = ctx.enter_context(tc.tile_pool(name="ps", bufs=1, space="PSUM"))

        # warm the ACT sqrt table while DMAs are in flight (exp's table loads
        # in the idle window between the sqrt and the first exp)
        junk = pool.tile([128, 1], f32, tag="junk")
        nc.scalar.activation(junk[:], const0[:, 0:1], Act.Sqrt)

        t_a = pool.tile([128, WA], f32, tag="ina")
        nc.sync.dma_start(t_a[:], g_a[:])
        t_b = pool.tile([J, WB], f32, tag="inb")
        nc.scalar.dma_start(t_b[:], g_b[:])

        ga3 = t_a[:, GA0 : GA0 + AL * 3].rearrange("p (a c) -> p a c", c=3)
        gb3 = t_a[:, GB0 : GB0 + 6].rearrange("p (h c) -> p h c", c=3)
        cb = t_a[:, CB0 : CB0 + NB]
        epsc = t_a[:, EPS0 : EPS0 + 1]

        # distance chain, split per chunk so both Sqrt activations retire
        # before the Exp table load starts: d[b, (ch, a)]
        d = pool.tile([128, 2 * AL], f32, tag="d")
        for ch in range(2):
            diff = pipe.tile([128, AL * 3], f32, tag="diff")
            diff3 = diff[:].rearrange("p (a c) -> p a c", c=3)
            nc.vector.tensor_sub(
                diff3,
                ga3,
                gb3[:, ch, :].unsqueeze(1).broadcast_to([128, AL, 3]),
            )
            sqd = pipe.tile([128, AL * 3], f32, tag="sqd")
            sqd3 = sqd[:].rearrange("p (a c) -> p a c", c=3)
            nc.vector.tensor_mul(sqd3, diff3, diff3)
            d2 = pipe.tile([128, AL], f32, tag="d2")
            nc.vector.tensor_reduce(
                d2[:], sqd3, axis=mybir.AxisListType.X, op=Alu.add
            )
            nc.scalar.activation(
                d[:, ch * AL : (ch + 1) * AL], d2[:], Act.Sqrt, bias=epsc
            )

        # m[b, (n,i)] per chunk, quartered so the first PSUM half is ready
        # early for its SBUF copy
        HW2 = NH * I  # 256
        pm = []
        for ch in range(2):
            p = ppool.tile([128, NB * I], f32, tag=f"pm{ch}", name=f"pm{ch}")
            for h in range(2):
                nc.tensor.matmul(
                    p[:, h * HW2 : (h + 1) * HW2],
                    lhsT=t_b[:, FT0 + ch * 128 : FT0 + (ch + 1) * 128],
                    rhs=t_b[:, WT0 + h * HW2 : WT0 + (h + 1) * HW2],
                    start=True,
                    stop=True,
                )
            pm.append(p)
        t_m = [
            pool.tile([128, NB * I], f32, tag=f"m{ch}", name=f"m{ch}")
            for ch in range(2)
        ]
        HW = NH * I  # columns per n-half = 256

        # rbf + contraction, pipelined in 4 units of (chunk, n-half).
        # PSUM->SBUF m copies are interleaved: each unit's m half is copied
        # just before it is needed, alternating DVE/ACT.
        po = ppool.tile([AL, I], f32, tag="po")
        first = True
        for ch in range(2):
            for h in range(2):
                u = ch * 2 + h
                tt = pipe.tile([128, NH * AL], f32, tag="tt")
                nc.vector.tensor_sub(
                    tt[:].rearrange("p (n a) -> p n a", n=NH),
                    d[:, ch * AL : (ch + 1) * AL]
                    .unsqueeze(1)
                    .broadcast_to([128, NH, AL]),
                    cb[:, h * NH : (h + 1) * NH]
                    .unsqueeze(2)
                    .broadcast_to([128, NH, AL]),
                )
                sq2 = pipe.tile([128, NH * AL], f32, tag="sq2")
                if u in (1, 3):
                    nc.gpsimd.tensor_mul(sq2[:], tt[:], tt[:])
                else:
                    nc.vector.tensor_mul(sq2[:], tt[:], tt[:])
                # copy this unit's m half on ACT just before its exp — ACT is
                # idle there, and DVE placement delayed the first contraction
                nc.scalar.copy(
                    t_m[ch][:, h * HW : (h + 1) * HW],
                    pm[ch][:, h * HW : (h + 1) * HW],
                )
                rbf = pipe.tile([128, NH * AL], f32, tag="rbf")
                nc.scalar.activation(rbf[:], sq2[:], Act.Exp, scale=-GAMMA)
                for k in range(NH):
                    n = h * NH + k
                    nc.tensor.matmul(
                        po[:],
                        lhsT=rbf[:, k * AL : (k + 1) * AL],
                        rhs=t_m[ch][:, n * I : (n + 1) * I],
                        start=first,
                        stop=(ch == 1 and n == NB - 1),
                    )
                    first = False
        t_o = pool.tile([AL, I], f32, tag="o")
        nc.vector.tensor_copy(t_o[:], po[:])
        nc.sync.dma_start(g_out[:], t_o[:])

    _split_multi_waits(nc)
    return nc


_NC = None


def _pack_inputs(features, geometry, centers, kernel_w, n_norm):
    features = np.asarray(features, np.float32)
    geometry = np.asarray(geometry, np.float32)
    centers = np.asarray(centers, np.float32)
    kernel_w = np.asarray(kernel_w, np.float32)
    scale = 1.0 / np.sqrt(float(np.asarray(n_norm).item()))

    wt = np.ascontiguousarray(kernel_w.transpose(2, 0, 1).reshape(J, NB * I))
    in_maps = []
    for core in range(NCORES):
        z, sl = divmod(core, AS)
        ina = np.empty((128, WA), np.float32)
        ina[:, GA0 : GA0 + AL * 3] = geometry[z, sl * AL : (sl + 1) * AL, :].reshape(
            1, AL * 3
        )
        ina[:, GB0 : GB0 + 6] = (
            geometry[z].reshape(2, 128, 3).transpose(1, 0, 2).reshape(128, 6)
        )
        ina[:, CB0 : CB0 + NB] = centers.reshape(1, NB)
        ina[:, EPS0] = EPS
        inb = np.empty((J, WB), np.float32)
        inb[:, FT0 : FT0 + P] = features[z].T * scale
        inb[:, WT0 : WT0 + NB * I] = wt
        in_maps.append({"ina": ina, "inb": inb})
    return in_maps


def kernel(features, geometry, centers, kernel_w, n_norm):
    global _NC
    if _NC is None:
        _NC = _build_program()

    in_maps = _pack_inputs(features, geometry, centers, kernel_w, n_norm)
    res = run_bass_kernel_spmd(_NC, in_maps, list(range(NCORES)))

    out = np.empty((B, P, I), np.float32)
    for core in range(NCORES):
        z, sl = divmod(core, AS)
        out[z, sl * AL : (sl + 1) * AL, :] = res.results[core]["out"]
    return out


# revision 13
# speedup vs baseline: 1.0515x; 1.0048x over previous
"""Trainium2 Bass kernel for continuous-filter convolution (gnn message passing).

Reference computation (shapes hardcoded):
    features [2,256,32] f32, geometry [2,256,3] f32, centers [16] f32,
    kernel_w [16,32,32] f32, n_norm scalar
    d[z,a,b]   = sqrt(sum_c (g[z,b,c]-g[z,a,c])^2 + 1e-9)
    rbf        = exp(-10*(d[...,None]-centers)^2)            [z,a,b,n]
    k          = einsum('zabn,nij->zabij', rbf, kernel_w) / sqrt(n_norm)
    out[z,a,i] = einsum('zabij,zbj->zai', k, features)

Key restructuring: k is never materialized. Instead
    m[z,b,n,i]  = sum_j kernel_w[n,i,j] * features[z,b,j] / sqrt(n_norm)
    out[z,a,i]  = sum_{b,n} rbf[z,a,b,n] * m[z,b,n,i]
which is a [64 x 4096] @ [4096 x 32] contraction per (z, a-slice).

Sharding: 8 cores = 2 batches x 4 a-slices of 64 points. Each core gets its
geometry slice plus replicated features/weights; no cross-device reduction.

Per-core dataflow (b = point index, 2 chunks of 128 on partitions):
    d[b,(ch,a)]  direct-form distances on DVE (keeps the a==b diagonal exact)
    rbf[b,(n,a)] per (chunk, n-half) unit: DVE sub -> DVE/GpSimd square ->
                 ACT exp -> 8 accumulating PE matmuls (K=128)
    m[b,(n,i)]   two PE matmuls vs pre-transposed weights; PSUM->SBUF copies
                 split between ACT and DVE per n-half, placed off the
                 critical chain
"""

import numpy as np
from contextlib import ExitStack

import concourse.bass as bass
import concourse.tile as tile
from concourse import mybir
from concourse.bass_utils import run_bass_kernel_spmd

GAMMA = 10.0
EPS = 1e-9
B, P, C = 2, 256, 32
NB, I, J = 16, 32, 32
NCORES = 8
AS = NCORES // B  # a-slices per batch = 4
AL = P // AS      # points per a-slice = 64
NH = NB // 2      # n-half size = 8

f32 = mybir.dt.float32

# packed input A: [128, 215] = gab(192, broadcast) | gb6(6) | cb(16) | eps(1)
GA0, GB0, CB0, EPS0 = 0, 192, 198, 214
WA = 215
# packed input B: [32, 768] = ft(256) | wt(512)
FT0, WT0 = 0, 256
WB = 768


def _split_multi_waits(nc):
    """This walrus build only lowers one sync wait per instruction; Tile's
    scheduler attaches several to some instructions (notably the tail drain).
    Hoist extras into single-wait EventSemaphore instructions just before, on
    the same engine — semantically identical, sequencer waits then issues."""
    n = 0
    for fn in nc.m.functions:
        for bb in fn.blocks:
            insts = list(bb.instructions)
            new = []
            for inst in insts:
                si = getattr(inst, "sync_info", None)
                if si is not None and si.on_wait and len(si.on_wait) > 1:
                    waits = list(si.on_wait)
                    for w in waits[:-1]:
                        n += 1
                        new.append(
                            mybir.InstEventSemaphore(
                                name=f"I-msplit{n}",
                                engine=inst.engine,
                                sync_info=mybir.SyncInfo(on_wait=[w], on_update=[]),
                            )
                        )
                    inst.sync_info = mybir.SyncInfo(
                        on_wait=[waits[-1]], on_update=list(si.on_update or [])
                    )
                new.append(inst)
            try:
                bb.instructions = new
            except Exception:
                bb.instructions.clear()
                for i in new:
                    bb.add_instruction(i)
    return n


def _build_program():
    nc = bass.Bass(debug=False)
    g_a = nc.declare_dram_parameter("ina", [128, WA], f32, isOutput=False)
    g_b = nc.declare_dram_parameter("inb", [J, WB], f32, isOutput=False)
    g_out = nc.declare_dram_parameter("out", [AL, I], f32, isOutput=True)

    Act = mybir.ActivationFunctionType
    Alu = mybir.AluOpType
    const0 = nc.const_aps.aps[(f32, 0.0)]

    with ExitStack() as ctx:
        tc = ctx.enter_context(tile.TileContext(nc))
        pool = ctx.enter_context(tc.tile_pool(name="sb", bufs=1))
        pipe = ctx.enter_context(tc.tile_pool(name="pipe", bufs=2))
        ppool = ctx.enter_context(tc.tile_pool(name="ps", bufs=1, space="PSUM"))

        # warm the ACT sqrt table while DMAs are in flight (exp's table loads
        # in the idle window between the sqrt and the first exp)
        junk = pool.tile([128, 1], f32, tag="junk")
        nc.scalar.activation(junk[:], const0[:, 0:1], Act.Sqrt)

        t_a = pool.tile([128, WA], f32, tag="ina")
        nc.sync.dma_start(t_a[:], g_a[:])
        t_b = pool.tile([J, WB], f32, tag="inb")
        nc.scalar.dma_start(t_b[:], g_b[:])

        ga3 = t_a[:, GA0 : GA0 + AL * 3].rearrange("p (a c) -> p a c", c=3)
        gb3 = t_a[:, GB0 : GB0 + 6].rearrange("p (h c) -> p h c", c=3)
        cb = t_a[:, CB0 : CB0 + NB]
        epsc = t_a[:, EPS0 : EPS0 + 1]

        # distance chain, split per chunk so both Sqrt activations retire
        # before the Exp table load starts: d[b, (ch, a)]
        d = pool.tile([128, 2 * AL], f32, tag="d")
        for ch in range(2):
            diff = pipe.tile([128, AL * 3], f32, tag="diff")
            diff3 = diff[:].rearrange("p (a c) -> p a c", c=3)
            nc.vector.tensor_sub(
                diff3,
                ga3,
                gb3[:, ch, :].unsqueeze(1).broadcast_to([128, AL, 3]),
            )
            sqd = pipe.tile([128, AL * 3], f32, tag="sqd")
            sqd3 = sqd[:].rearrange("p (a c) -> p a c", c=3)
            nc.vector.tensor_mul(sqd3, diff3, diff3)
            d2 = pipe.tile([128, AL], f32, tag="d2")
            nc.vector.tensor_reduce(
                d2[:], sqd3, axis=mybir.AxisListType.X, op=Alu.add
            )
            nc.scalar.activation(
                d[:, ch * AL : (ch + 1) * AL], d2[:], Act.Sqrt, bias=epsc
            )

        # m[b, (n,i)] per chunk, quartered so the first PSUM half is ready
        # early for its SBUF copy
        HW2 = NH * I  # 256
        pm = []
        for ch in range(2):
            p = ppool.tile([128, NB * I], f32, tag=f"pm{ch}", name=f"pm{ch}")
            for h in range(2):
                nc.tensor.matmul(
                    p[:, h * HW2 : (h + 1) * HW2],
                    lhsT=t_b[:, FT0 + ch * 128 : FT0 + (ch + 1) * 128],
                    rhs=t_b[:, WT0 + h * HW2 : WT0 + (h + 1) * HW2],
                    start=True,
                    stop=True,
                )
            pm.append(p)
        t_m = [
            pool.tile([128, NB * I], f32, tag=f"m{ch}", name=f"m{ch}")
            for ch in range(2)
        ]
        HW = NH * I  # columns per n-half = 256

        # rbf + contraction, pipelined in 4 units of (chunk, n-half).
        # PSUM->SBUF m copies are interleaved: each unit's m half is copied
        # just before it is needed, alternating DVE/ACT.
        po = ppool.tile([AL, I], f32, tag="po")
        first = True
        for ch in range(2):
            for h in range(2):
                u = ch * 2 + h
                tt = pipe.tile([128, NH * AL], f32, tag="tt")
                nc.vector.tensor_sub(
                    tt[:].rearrange("p (n a) -> p n a", n=NH),
                    d[:, ch * AL : (ch + 1) * AL]
                    .unsqueeze(1)
                    .broadcast_to([128, NH, AL]),
                    cb[:, h * NH : (h + 1) * NH]
                    .unsqueeze(2)
                    .broadcast_to([128, NH, AL]),
                )
                sq2 = pipe.tile([128, NH * AL], f32, tag="sq2")
                if u == 1:
                    nc.gpsimd.tensor_mul(sq2[:], tt[:], tt[:])
                else:
                    nc.vector.tensor_mul(sq2[:], tt[:], tt[:])
                # copy this unit's m half on ACT just before its exp — ACT is
                # idle there, and DVE placement delayed the first contraction
                nc.scalar.copy(
                    t_m[ch][:, h * HW : (h + 1) * HW],
                    pm[ch][:, h * HW : (h + 1) * HW],
                )
                rbf = pipe.tile([128, NH * AL], f32, tag="rbf")
                nc.scalar.activation(rbf[:], sq2[:], Act.Exp, scale=-GAMMA)
                for k in range(NH):
                    n = h * NH + k
                    nc.tensor.matmul(
                        po[:],
                        lhsT=rbf[:, k * AL : (k + 1) * AL],
                        rhs=t_m[ch][:, n * I : (n + 1) * I],
                        start=first,
                        stop=(ch == 1 and n == NB - 1),
                    )
                    first = False
        t_o = pool.tile([AL, I], f32, tag="o")
        nc.scalar.copy(t_o[:], po[:])
        nc.sync.dma_start(g_out[:], t_o[:])

    _split_multi_waits(nc)
    return nc


_NC = None


def _pack_inputs(features, geometry, centers, kernel_w, n_norm):
    features = np.asarray(features, np.float32)
    geometry = np.asarray(geometry, np.float32)
    centers = np.asarray(centers, np.float32)
    kernel_w = np.asarray(kernel_w, np.float32)
    scale = 1.0 / np.sqrt(float(np.asarray(n_norm).item()))

    wt = np.ascontiguousarray(kernel_w.transpose(2, 0, 1).reshape(J, NB * I))
    in_maps = []
    for core in range(NCORES):
        z, sl = divmod(core, AS)
        ina = np.empty((128, WA), np.float32)
        ina[:, GA0 : GA0 + AL * 3] = geometry[z, sl * AL : (sl + 1) * AL, :].reshape(
            1, AL * 3
        )
        ina[:, GB0 : GB0 + 6] = (
            geometry[z].reshape(2, 128, 3).transpose(1, 0, 2).reshape(128, 6)
        )
        ina[:, CB0 : CB0 + NB] = centers.reshape(1, NB)
        ina[:, EPS0] = EPS
        inb = np.empty((J, WB), np.float32)
        inb[:, FT0 : FT0 + P] = features[z].T * scale
        inb[:, WT0 : WT0 + NB * I] = wt
        in_maps.append({"ina": ina, "inb": inb})
    return in_maps


def kernel(features, geometry, centers, kernel_w, n_norm):
    global _NC
    if _NC is None:
        _NC = _build_program()

    in_maps = _pack_inputs(features, geometry, centers, kernel_w, n_norm)
    res = run_bass_kernel_spmd(_NC, in_maps, list(range(NCORES)))

    out = np.empty((B, P, I), np.float32)
    for core in range(NCORES):
        z, sl = divmod(core, AS)
        out[z, sl * AL : (sl + 1) * AL, :] = res.results[core]["out"]
    return out
